# revision 14
# baseline (speedup 1.0000x reference)
"""GraphSAGE 2-layer (SAGEConv mean-aggregation) Bass kernel for 8 TRN2 NeuronCores.

Strategy (see spec sharding_hint):
  - Destination nodes sharded across 8 cores (12500/core). Within each core a
    greedy balancer assigns nodes to 98 windows x 128 slots so that each
    (window, src-block) cell has <= ~512 edges -> near-uniform SPMD schedule.
  - Edges partitioned by destination core, sorted by (window, src-block).
  - Aggregation: dma_gather pulls x[src] rows (bf16) from HBM in 4 source
    blocks of 25000 rows (int16 index limit); VectorE builds one-hot*invdeg
    selection tiles via fused tensor_scalar(is_equal, mult) against an iota
    constant; TensorE accumulates mean^T per window in PSUM (K=128 edges per
    chunk matmul).
  - Transform per window: two 128x128 matmuls (+ rank-1 bias matmul / ACT
    bias) produce h; layer-1 h rows are scattered back into per-core shard
    (dma_scatter_add into zeroed DRAM), AllGather forms full h for layer 2.
  - Final layer-2 output is written in slot order and inverse-permuted on
    host.
"""

import sys

sys.path.insert(0, "/opt/trn_rl_repo")

from contextlib import ExitStack
from dataclasses import dataclass

import ml_dtypes
import numpy as np

import concourse.bacc as bacc
import concourse.bass as bass
import concourse.mybir as mybir
import concourse.tile as tile
from concourse.bass_utils import run_bass_kernel_spmd

BF = mybir.dt.bfloat16
F32 = mybir.dt.float32
I16 = mybir.dt.int16
bfnp = ml_dtypes.bfloat16


@dataclass
class Cfg:
    N: int = 100000      # total nodes
    D: int = 128         # feature dim
    C: int = 8           # cores
    NB: int = 4          # source blocks (int16 gather index limit)
    WN: int = 98         # windows per core (128 dst nodes each)
    CALL: int = 1024     # gather indices per dma_gather call
    SCW: int = 8         # windows per scatter call / out-dma (<= CALL//128)
    OCW: int = 8         # windows per final output dma

    @property
    def NSH(self):
        return self.N // self.C

    @property
    def BS(self):
        return self.N // self.NB

    @property
    def SLOTS(self):
        return self.WN * 128

    @property
    def TRASH(self):
        return 128

    @property
    def CALLCH(self):
        return self.CALL // 128


CFG = Cfg()


# ---------------------------------------------------------------- host prep


def _balance_core(dnb, WN, cap=128, ctarget=512):
    """Assign nodes (rows of dnb, per-block in-degree vectors) to WN bins of
    <=cap nodes, aiming for per-(bin, block) sums <= target. Overflow (when a
    block's total exceeds WN*ctarget) is concentrated in the LAST windows.
    Returns (bin id per node, binsum)."""
    nn, NB = dnb.shape
    T = dnb.sum(0)
    # per-block overflow chunks, assigned to tail windows
    target = np.full((WN, NB), ctarget, np.int64)
    for b in range(NB):
        q = max(0, -(-int(T[b] - WN * ctarget) // 128))
        for i in range(min(q, WN)):
            target[WN - 1 - i, b] += 128
    tot = dnb.sum(1)
    order = np.argsort(-tot, kind="stable")
    binsum = np.zeros((WN, NB), np.int64)
    binslots = np.zeros(WN, np.int64)
    assign = np.full(nn, -1, np.int64)
    tgt = target.astype(np.float64)
    for n in order:
        dv = dnb[n]
        fill = ((binsum + dv) / tgt).max(axis=1)
        fill += 1e-5 * binslots
        fill[binslots >= cap] = 1e30
        j = int(np.argmin(fill))
        assign[n] = j
        binsum[j] += dv
        binslots[j] += 1

    # repair: push cells over target down via node moves/swaps
    for _ in range(10):
        viol = np.argwhere(binsum > target)
        if len(viol) == 0:
            break
        moved = 0
        for j, b in viol:
            over = binsum[j, b] - target[j, b]
            if over <= 0:
                continue
            members = np.where(assign == j)[0]
            members = members[np.argsort(-dnb[members, b], kind="stable")]
            for n in members:
                if over <= 0:
                    break
                dv = dnb[n]
                if dv[b] == 0:
                    break
                # move to a bin with room where all blocks stay <= target
                ok = ((binsum + dv) <= target).all(axis=1) & (binslots < cap)
                ok[j] = False
                cand = np.where(ok)[0]
                if len(cand):
                    j2 = int(cand[np.argmin((binsum[cand] + dv).max(1))])
                    assign[n] = j2
                    binsum[j] -= dv
                    binsum[j2] += dv
                    binslots[j] -= 1
                    binslots[j2] += 1
                    over = binsum[j, b] - target[j, b]
                    moved += 1
                    continue
                # swap with a lighter node elsewhere
                done = False
                for j2 in np.argsort(binsum[:, b]):
                    if j2 == j:
                        continue
                    mem2 = np.where(assign == j2)[0]
                    if len(mem2) == 0:
                        continue
                    m = mem2[np.argmin(dnb[mem2, b])]
                    dm = dnb[m]
                    if dm[b] >= dv[b]:
                        continue
                    nj = binsum[j] - dv + dm
                    nj2 = binsum[j2] - dm + dv
                    if (nj <= target[j]).all() and (nj2 <= target[j2]).all():
                        assign[n], assign[m] = j2, j
                        binsum[j] = nj
                        binsum[j2] = nj2
                        over = binsum[j, b] - target[j, b]
                        moved += 1
                        done = True
                        break
                if not done:
                    break
        if moved == 0:
            break
    return assign, binsum


def prep(x, edge_index, cfg=CFG):
    """Host-side sharding/schedule. Returns (schedule, per-core input maps,
    host metadata for unsharding)."""
    C, NB, WN, NSH, BS, CALL = cfg.C, cfg.NB, cfg.WN, cfg.NSH, cfg.BS, cfg.CALL
    src = np.asarray(edge_index[0]).astype(np.int64)
    dst = np.asarray(edge_index[1]).astype(np.int64)
    E = src.shape[0]

    deg = np.bincount(dst, minlength=cfg.N).astype(np.float64)
    invdeg = (1.0 / np.maximum(deg, 1.0)).astype(np.float32)
    vedge_all = invdeg[dst].astype(bfnp)

    ecore = dst // NSH
    eblock = src // BS

    # --- per-core balance: node-local id -> (window, pos)
    win_of = np.zeros(cfg.N, np.int64)   # window within core
    pos_of = np.zeros(cfg.N, np.int64)   # slot within window
    counts = np.zeros((C, WN, NB), np.int64)
    for c in range(C):
        lo = c * NSH
        dnb = np.zeros((NSH, NB), np.int64)
        emask = ecore == c
        np.add.at(dnb, (dst[emask] - lo, eblock[emask]), 1)
        assign, binsum = _balance_core(dnb, WN)
        # order bins by descending per-block chunk tuple so heavy cells align
        # at the same window index across cores
        kt = np.ceil(binsum / 128).astype(np.int64)
        key = [tuple(-kt[j]) + tuple(-binsum[j]) for j in range(WN)]
        order = sorted(range(WN), key=lambda j: key[j])
        rank = np.empty(WN, np.int64)
        rank[order] = np.arange(WN)
        w = rank[assign]
        win_of[lo:lo + NSH] = w
        # position within window: stable by node id
        order2 = np.lexsort((np.arange(NSH), w))
        pos = np.zeros(NSH, np.int64)
        pcount = np.zeros(WN, np.int64)
        for m in order2:
            pos[m] = pcount[w[m]]
            pcount[w[m]] += 1
        pos_of[lo:lo + NSH] = pos
        cnt = np.zeros((WN, NB), np.int64)
        np.add.at(cnt, (w[dst[emask] - lo], eblock[emask]), 1)
        counts[c] = cnt

    K = np.ceil(counts / 128).astype(np.int64).max(axis=0)  # [WN, NB]
    TCH = int(K.sum())  # chunks per core per layer

    # per-block stream lengths and call counts (uniform across cores)
    Sb = (K.sum(axis=0) * 128).astype(np.int64)             # [NB] idx slots
    ncalls = np.ceil(Sb / CALL).astype(np.int64)
    lastvalid = Sb - (ncalls - 1) * CALL                     # valid idxs in last call

    ewin = win_of[dst]
    epos = pos_of[dst]

    in_maps = []
    for c in range(C):
        lo = c * NSH
        emask = ecore == c
        es, ed = src[emask], dst[emask]
        ew, eb = ewin[emask], eblock[emask]
        ep = epos[emask]
        ev = vedge_all[emask]
        okey = np.lexsort((np.arange(es.shape[0]), eb, ew))
        es, ed, ew, eb, ep, ev = (a[okey] for a in (es, ed, ew, eb, ep, ev))
        cellcnt = counts[c]

        gstreams = [np.zeros(int(ncalls[b]) * CALL, np.int16) for b in range(NB)]
        for b in range(NB):
            if Sb[b] < ncalls[b] * CALL:
                gstreams[b][Sb[b]:] = -1  # tail of last call: skipped by DMA
        sval = np.zeros((TCH, 128, 128), bfnp)  # per-chunk one-hot*invdeg
        drel = np.full((TCH, 128), 255.0, np.float32)
        vval = np.zeros((TCH, 128), np.float32)

        # walk cells in (w, b) order; edges already sorted that way
        eoff = 0
        gcol = 0
        posb = np.zeros(NB, np.int64)
        for w in range(WN):
            for b in range(NB):
                L = int(cellcnt[w, b])
                kwb = int(K[w, b])
                if kwb == 0:
                    assert L == 0
                    continue
                sl = slice(eoff, eoff + L)
                st = int(posb[b])
                gstreams[b][st:st + L] = (es[sl] - b * BS).astype(np.int16)
                # pads within the cell keep idx 0 (gather row 0, killed by
                # one-hot 255)
                ar = np.arange(L)
                sval[gcol + ar // 128, ar % 128, ep[sl]] = ev[sl]
                drel.reshape(-1)[gcol * 128:gcol * 128 + L] = ep[sl]
                vval.reshape(-1)[gcol * 128:gcol * 128 + L] = ev[sl].astype(np.float32)
                posb[b] += kwb * 128
                gcol += kwb
                eoff += L
        assert eoff == es.shape[0]
        assert gcol == TCH

        # wrap gather idx streams into [128, cols] int16 (16-partition wrap,
        # replicated x8)
        gparts = []
        for b in range(NB):
            arr = gstreams[b].reshape(-1, 16).T  # [16, Sb_pad/16]
            gparts.append(arr)
        gidx16 = np.concatenate(gparts, axis=1)          # [16, GCOLS]
        gidx = np.tile(gidx16, (8, 1)).astype(np.int16)  # [128, GCOLS]

        # scatter indices: slot s=(w*128+p) -> node-local row, dummies to trash
        sl_nodes = np.full(cfg.SLOTS, -1, np.int64)
        msk = np.arange(cfg.N)[lo:lo + NSH]
        sl_idx = win_of[msk] * 128 + pos_of[msk]
        sl_nodes[sl_idx] = np.arange(NSH)
        trash = cfg.NSH + (np.arange(cfg.SLOTS) % cfg.TRASH)
        starg = np.where(sl_nodes >= 0, sl_nodes, trash).astype(np.int16)
        sidx16 = starg.reshape(-1, 16).T
        sidx = np.tile(sidx16, (8, 1)).astype(np.int16)

        # xT in slot order
        xT = np.zeros((cfg.D, cfg.SLOTS), bfnp)
        xT[:, sl_idx] = np.asarray(x[lo:lo + NSH]).astype(bfnp).T

        is_dve = (np.arange(TCH) % 5) < 2      # 40% DVE-built
        sv = sval[~is_dve]
        in_maps.append(dict(
            gidx=gidx, sidx=sidx,
            sval=np.ascontiguousarray(sv.transpose(1, 0, 2).reshape(128, -1)),
            dstrel=np.ascontiguousarray(drel.T), vval=np.ascontiguousarray(vval.T),
            xT=np.ascontiguousarray(xT),
            slot_nodes=sl_nodes,                   # host-only
        ))

    is_dve = (np.arange(TCH) % 5) < 2
    sidx_of = np.cumsum(~is_dve) - 1              # streamed-chunk index
    sched = dict(K=K, TCH=TCH, ncalls=ncalls, lastvalid=lastvalid, Sb=Sb,
                 is_dve=is_dve, sidx_of=sidx_of, NSV=int((~is_dve).sum()))
    return sched, in_maps


# ---------------------------------------------------------------- program


def build(cfg, sched):
    K, TCH = sched["K"], sched["TCH"]
    ncalls, lastvalid = sched["ncalls"], sched["lastvalid"]
    C, D, NB, WN, NSH, BS = cfg.C, cfg.D, cfg.NB, cfg.WN, cfg.NSH, cfg.BS
    CALL, CALLCH, SLOTS = cfg.CALL, cfg.CALLCH, cfg.SLOTS
    GCOLS_B = [int(ncalls[b]) * (CALL // 16) for b in range(NB)]
    GOFF = np.concatenate([[0], np.cumsum(GCOLS_B)]).astype(int)

    nc = bacc.Bacc(None, num_devices=C, num_swdge_queues=4,
                   dynamic_dma_scratch_size=32768)
    x_d = nc.dram_tensor("xbf", [cfg.N, D], BF, kind="ExternalInput")
    xT_d = nc.dram_tensor("xT", [D, SLOTS], BF, kind="ExternalInput")
    gidx_d = nc.dram_tensor("gidx", [128, int(GOFF[-1])], I16, kind="ExternalInput")
    sidx_d = nc.dram_tensor("sidx", [128, SLOTS // 16], I16, kind="ExternalInput")
    is_dve, sidx_of, NSV = sched["is_dve"], sched["sidx_of"], sched["NSV"]
    sval_d = nc.dram_tensor("sval", [128, NSV * 128], BF, kind="ExternalInput")
    drel_d = nc.dram_tensor("dstrel", [128, TCH], F32, kind="ExternalInput")
    vval_d = nc.dram_tensor("vval", [128, TCH], F32, kind="ExternalInput")
    w_d = {}
    for nm in ("wlt1", "wrt1", "wlt2", "wrt2"):
        w_d[nm] = nc.dram_tensor(nm, [D, D], BF, kind="ExternalInput")
    b1_d = nc.dram_tensor("b1c", [D, 1], F32, kind="ExternalInput")
    b2_d = nc.dram_tensor("b2r", [1, D], F32, kind="ExternalInput")
    out_d = nc.dram_tensor("out", [SLOTS, D], F32, kind="ExternalOutput")

    ident_d = nc.inline_tensor(np.eye(128, dtype=bfnp), "identc")
    iota_d = nc.inline_tensor(
        np.broadcast_to(np.arange(128, dtype=bfnp), (128, 128)).copy(), "iotac")
    ones_d = nc.inline_tensor(np.ones((1, 128), np.float32), "onesc")

    hsh_d = nc.dram_tensor("hshard", [NSH + cfg.TRASH, D], BF)  # Internal
    hfull_d = nc.dram_tensor("hfull", [cfg.N, D], BF, addr_space="Shared")

    with tile.TileContext(nc) as tc, ExitStack() as ctx:
        const = ctx.enter_context(tc.tile_pool(name="const", bufs=1))
        meta = ctx.enter_context(tc.tile_pool(name="meta", bufs=1))
        gpool = ctx.enter_context(tc.tile_pool(name="gather", bufs=6))
        spool = ctx.enter_context(tc.tile_pool(name="sv", bufs=6))
        ohp = ctx.enter_context(tc.tile_pool(name="oh", bufs=8))
        mwp = ctx.enter_context(tc.tile_pool(name="mw", bufs=4))
        htp = ctx.enter_context(tc.tile_pool(name="ht", bufs=1))
        xtp = ctx.enter_context(tc.tile_pool(name="xt", bufs=1))
        stgp = ctx.enter_context(tc.tile_pool(name="stg", bufs=2))
        psA = ctx.enter_context(tc.tile_pool(name="psA", bufs=3, space="PSUM"))
        psB = ctx.enter_context(tc.tile_pool(name="psB", bufs=2, space="PSUM"))
        psT = ctx.enter_context(tc.tile_pool(name="psT", bufs=2, space="PSUM"))

        def load(pool, dram, shape, dtype):
            t = pool.tile(shape, dtype, tag=dram.name)
            nc.sync.dma_start(t[:], dram[:])
            return t

        ident_s = load(const, ident_d, [128, 128], BF)
        iota_s = load(const, iota_d, [128, 128], BF)
        drel_s = load(meta, drel_d, [128, TCH], F32)
        vval_s = load(meta, vval_d, [128, TCH], F32)
        ones_s = load(const, ones_d, [1, 128], F32)
        w_s = {nm: load(const, w_d[nm], [D, D], BF) for nm in w_d}
        b1_s = load(const, b1_d, [D, 1], F32)
        b2_s = load(const, b2_d, [1, D], F32)

        gidx_s = load(meta, gidx_d, [128, int(GOFF[-1])], I16)
        sidx_s = load(meta, sidx_d, [128, SLOTS // 16], I16)
        xT_s = load(xtp, xT_d, [D, SLOTS], BF)
        hT_s = htp.tile([D, SLOTS], BF, tag="hT")

        # zero h_shard
        zt = const.tile([128, 1024], BF, tag="zeros")
        nc.vector.memset(zt[:], 0.0)
        nrows_total = NSH + cfg.TRASH
        r0 = 0
        while r0 < nrows_total:
            nr = min(1024, nrows_total - r0)
            nc.sync.dma_start(hsh_d[r0:r0 + nr, :], zt[:, :nr])
            r0 += nr

        def run_layer(L):
            src_d = x_d if L == 1 else hfull_d
            posb = [0] * NB
            gt = [None] * NB
            stg_tile = None
            ostg_tile = None
            SCW, OCW = cfg.SCW, cfg.OCW
            for w in range(WN):
                nchunks_w = int(K[w].sum())
                psum_a = psA.tile([128, 128], F32, tag="agg")
                ci = 0
                for b in range(NB):
                    for k in range(int(K[w, b])):
                        pos = posb[b]
                        call_i, col = divmod(pos, CALLCH)
                        if col == 0:
                            gt[b] = gpool.tile([128, CALLCH, 128], BF, tag=f"g{b}",
                                               name=f"g{b}_{L}_{call_i}")
                            nvalid = CALL if call_i < int(ncalls[b]) - 1 else int(lastvalid[b])
                            ioff = GOFF[b] + call_i * (CALL // 16)
                            nc.gpsimd.dma_gather(
                                out_ap=gt[b][:],
                                in_ap=src_d[b * BS:(b + 1) * BS, :],
                                idxs_ap=gidx_s[:, ioff:ioff + CALL // 16],
                                num_idxs=CALL,
                                num_idxs_reg=nvalid,
                                elem_size=D,
                            )
                        gcol = run_layer.gcol
                        if is_dve[gcol]:
                            S = ohp.tile([128, 128], BF, tag="oh",
                                         name=f"oh{L}_{gcol}")
                            nc.vector.tensor_scalar(
                                out=S[:], in0=iota_s[:],
                                scalar1=drel_s[:, gcol:gcol + 1],
                                scalar2=vval_s[:, gcol:gcol + 1],
                                op0=mybir.AluOpType.is_equal,
                                op1=mybir.AluOpType.mult,
                            )
                            rhs_ap = S[:]
                        else:
                            si, sc = divmod(int(sidx_of[gcol]), 8)
                            if sc == 0:
                                nch = min(8, NSV - si * 8)
                                st = spool.tile([128, 8, 128], BF, tag="sv",
                                                name=f"sv{L}_{si}")
                                nc.sync.dma_start(
                                    st[:, :nch, :],
                                    sval_d[:, si * 1024:si * 1024 + nch * 128])
                                run_layer.stile = st
                            rhs_ap = run_layer.stile[:, sc, :]
                        nc.tensor.matmul(
                            out=psum_a[:], lhsT=gt[b][:, col, :], rhs=rhs_ap,
                            start=(ci == 0), stop=(ci == nchunks_w - 1),
                        )
                        run_layer.gcol += 1
                        posb[b] += 1
                        ci += 1
                m_s = mwp.tile([128, 128], BF, tag="mw")
                if nchunks_w:
                    nc.vector.tensor_copy(m_s[:], psum_a[:])
                else:
                    nc.vector.memset(m_s[:], 0.0)
                wsl = slice(w * 128, (w + 1) * 128)
                if L == 1:
                    psum_h = psB.tile([128, 128], F32, tag="h")
                    nc.tensor.matmul(out=psum_h[:], lhsT=w_s["wlt1"][:], rhs=m_s[:],
                                     start=True, stop=False)
                    nc.tensor.matmul(out=psum_h[:], lhsT=w_s["wrt1"][:],
                                     rhs=xT_s[:, wsl], start=False, stop=True)
                    nc.scalar.activation(hT_s[:, wsl], psum_h[:],
                                         mybir.ActivationFunctionType.Identity,
                                         bias=b1_s[:, 0:1], scale=1.0)
                    psum_t = psT.tile([128, 128], BF, tag="tr")
                    nc.tensor.transpose(psum_t[:], hT_s[:, wsl], ident_s[:])
                    wi = w % SCW
                    if wi == 0:
                        stg_tile = stgp.tile([128, SCW, 128], BF, tag="stg")
                    nc.vector.tensor_copy(stg_tile[:, wi, :], psum_t[:])
                    if wi == SCW - 1 or w == WN - 1:
                        used = wi + 1
                        w0 = w - wi
                        nc.gpsimd.dma_scatter_add(
                            out_ap=hsh_d[:, :],
                            in_ap=stg_tile[:, :used, :],
                            idxs_ap=sidx_s[:, w0 * 8:(w0 + used) * 8],
                            num_idxs=used * 128,
                            num_idxs_reg=used * 128,
                            elem_size=D,
                        )
                else:
                    psum_h = psB.tile([128, 128], F32, tag="h")
                    nc.tensor.matmul(out=psum_h[:], lhsT=m_s[:], rhs=w_s["wlt2"][:],
                                     start=True, stop=False)
                    nc.tensor.matmul(out=psum_h[:], lhsT=hT_s[:, wsl],
                                     rhs=w_s["wrt2"][:], start=False, stop=False)
                    nc.tensor.matmul(out=psum_h[:], lhsT=ones_s[0:1, :],
                                     rhs=b2_s[0:1, :], start=False, stop=True)
                    wi = w % OCW
                    if wi == 0:
                        ostg_tile = stgp.tile([128, OCW, 128], F32, tag="ostg")
                    nc.vector.tensor_copy(ostg_tile[:, wi, :], psum_h[:])
                    if wi == OCW - 1 or w == WN - 1:
                        used = wi + 1
                        w0 = w - wi
                        oap = out_d[:].rearrange("(w p) f -> p w f", p=128)
                        nc.sync.dma_start(oap[:, w0:w0 + used, :],
                                          ostg_tile[:, :used, :])

        run_layer.gcol = 0
        run_layer.stile = None
        run_layer(1)
        nc.gpsimd.collective_compute(
            "AllGather", mybir.AluOpType.bypass,
            replica_groups=[list(range(C))],
            ins=[hsh_d[0:NSH, :]],
            outs=[hfull_d[:]],
        )
        run_layer.gcol = 0
        run_layer.stile = None
        run_layer(2)

    # spread SWDGE gather/scatter descriptor generation across the 4 SWDGE
    # queues (parallel Q7 pairs). Tile assigned DMASW lanes round-robin in
    # scheduled order; keep sem-lane <-> queue binding consistent by deriving
    # the queue from the lane (lane % 4).
    from concourse.tile_sem_assignment import PROC_NAME_TO_IDX
    dmasw0 = PROC_NAME_TO_IDX["DMASW0"]
    for inst in nc.inst_map.values():
        if isinstance(inst, (mybir.InstDMAGatherAnt, mybir.InstDMAScatterAddAnt)):
            proc = getattr(inst, "bass_scheduled_proc", None)
            if proc is not None and dmasw0 <= proc < dmasw0 + 8:
                inst.queue_num = (proc - dmasw0) % 4

    nc.compile()
    return nc


# ---------------------------------------------------------------- kernel


def kernel(**inputs):
    cfg = CFG
    x = np.asarray(inputs["x"], np.float32)
    ei = np.asarray(inputs["edge_index"])
    sched, in_maps = prep(x, ei, cfg)
    nc = build(cfg, sched)

    x_bf = x.astype(bfnp)
    shared = dict(
        xbf=x_bf,
        wlt1=np.ascontiguousarray(np.asarray(inputs["Wl1"], np.float32).T.astype(bfnp)),
        wrt1=np.ascontiguousarray(np.asarray(inputs["Wr1"], np.float32).T.astype(bfnp)),
        wlt2=np.ascontiguousarray(np.asarray(inputs["Wl2"], np.float32).T.astype(bfnp)),
        wrt2=np.ascontiguousarray(np.asarray(inputs["Wr2"], np.float32).T.astype(bfnp)),
        b1c=np.asarray(inputs["b1"], np.float32).reshape(cfg.D, 1).copy(),
        b2r=np.asarray(inputs["b2"], np.float32).reshape(1, cfg.D).copy(),
    )
    slot_nodes = [m.pop("slot_nodes") for m in in_maps]
    run_maps = [dict(shared, **{k: v for k, v in m.items()}) for m in in_maps]

    res = run_bass_kernel_spmd(nc, run_maps, core_ids=list(range(cfg.C)))
    out = np.empty((cfg.N, cfg.D), np.float32)
    for c in range(cfg.C):
        oc = res.results[c]["out"]
        sn = slot_nodes[c]
        real = sn >= 0
        out[c * cfg.NSH + sn[real]] = oc[real]
    return out


if __name__ == "__main__":
    d = np.load("/tmp/inputs.npz")
    ins = {k: d[k] for k in ("x", "edge_index", "Wl1", "Wr1", "b1", "Wl2", "Wr2", "b2")}
    got = kernel(**ins)
    exp = d["expected"]
    err = np.abs(got - exp).max() / np.abs(exp).max()
    print("Relative error:", err)


# revision 15
# speedup vs baseline: 1.3105x; 1.3105x over previous
"""GraphSAGE 2-layer (SAGEConv mean-aggregation) Bass kernel for 8 TRN2 NeuronCores.

Strategy (see spec sharding_hint):
  - Destination nodes sharded across 8 cores (12500/core). Within each core a
    greedy balancer assigns nodes to 98 windows x 128 slots so that each
    (window, src-block) cell has <= ~512 edges -> near-uniform SPMD schedule.
  - Edges partitioned by destination core, sorted by (window, src-block).
  - Aggregation: dma_gather pulls x[src] rows (bf16) from HBM in 4 source
    blocks of 25000 rows (int16 index limit); VectorE builds one-hot*invdeg
    selection tiles via fused tensor_scalar(is_equal, mult) against an iota
    constant; TensorE accumulates mean^T per window in PSUM (K=128 edges per
    chunk matmul).
  - Transform per window: two 128x128 matmuls (+ rank-1 bias matmul / ACT
    bias) produce h; layer-1 h rows are scattered back into per-core shard
    (dma_scatter_add into zeroed DRAM), AllGather forms full h for layer 2.
  - Final layer-2 output is written in slot order and inverse-permuted on
    host.
"""

import sys

sys.path.insert(0, "/opt/trn_rl_repo")

from contextlib import ExitStack
from dataclasses import dataclass

import ml_dtypes
import numpy as np

import concourse.bacc as bacc
import concourse.bass as bass
import concourse.mybir as mybir
import concourse.tile as tile
from concourse.bass_utils import run_bass_kernel_spmd

BF = mybir.dt.bfloat16
F32 = mybir.dt.float32
I16 = mybir.dt.int16
bfnp = ml_dtypes.bfloat16


@dataclass
class Cfg:
    N: int = 100000      # total nodes
    D: int = 128         # feature dim
    C: int = 8           # cores
    NB: int = 4          # source blocks (int16 gather index limit)
    WN: int = 98         # windows per core (128 dst nodes each)
    CALL: int = 1024     # gather indices per dma_gather call
    SCW: int = 8         # windows per scatter call / out-dma (<= CALL//128)
    OCW: int = 8         # windows per final output dma

    @property
    def NSH(self):
        return self.N // self.C

    @property
    def BS(self):
        return self.N // self.NB

    @property
    def SLOTS(self):
        return self.WN * 128

    @property
    def TRASH(self):
        return 128

    @property
    def CALLCH(self):
        return self.CALL // 128


CFG = Cfg()


# ---------------------------------------------------------------- host prep


def _balance_core(dnb, WN, cap=128, ctarget=512):
    """Assign nodes (rows of dnb, per-block in-degree vectors) to WN bins of
    <=cap nodes, aiming for per-(bin, block) sums <= target. Overflow (when a
    block's total exceeds WN*ctarget) is concentrated in the LAST windows.
    Returns (bin id per node, binsum)."""
    nn, NB = dnb.shape
    T = dnb.sum(0)
    # per-block overflow chunks, assigned to tail windows
    target = np.full((WN, NB), ctarget, np.int64)
    for b in range(NB):
        q = max(0, -(-int(T[b] - WN * ctarget) // 128))
        for i in range(min(q, WN)):
            target[WN - 1 - i, b] += 128
    tot = dnb.sum(1)
    order = np.argsort(-tot, kind="stable")
    binsum = np.zeros((WN, NB), np.int64)
    binslots = np.zeros(WN, np.int64)
    assign = np.full(nn, -1, np.int64)
    tgt = target.astype(np.float64)
    for n in order:
        dv = dnb[n]
        fill = ((binsum + dv) / tgt).max(axis=1)
        fill += 1e-5 * binslots
        fill[binslots >= cap] = 1e30
        j = int(np.argmin(fill))
        assign[n] = j
        binsum[j] += dv
        binslots[j] += 1

    # repair: push cells over target down via node moves/swaps
    for _ in range(10):
        viol = np.argwhere(binsum > target)
        if len(viol) == 0:
            break
        moved = 0
        for j, b in viol:
            over = binsum[j, b] - target[j, b]
            if over <= 0:
                continue
            members = np.where(assign == j)[0]
            members = members[np.argsort(-dnb[members, b], kind="stable")]
            for n in members:
                if over <= 0:
                    break
                dv = dnb[n]
                if dv[b] == 0:
                    break
                # move to a bin with room where all blocks stay <= target
                ok = ((binsum + dv) <= target).all(axis=1) & (binslots < cap)
                ok[j] = False
                cand = np.where(ok)[0]
                if len(cand):
                    j2 = int(cand[np.argmin((binsum[cand] + dv).max(1))])
                    assign[n] = j2
                    binsum[j] -= dv
                    binsum[j2] += dv
                    binslots[j] -= 1
                    binslots[j2] += 1
                    over = binsum[j, b] - target[j, b]
                    moved += 1
                    continue
                # swap with a lighter node elsewhere
                done = False
                for j2 in np.argsort(binsum[:, b]):
                    if j2 == j:
                        continue
                    mem2 = np.where(assign == j2)[0]
                    if len(mem2) == 0:
                        continue
                    m = mem2[np.argmin(dnb[mem2, b])]
                    dm = dnb[m]
                    if dm[b] >= dv[b]:
                        continue
                    nj = binsum[j] - dv + dm
                    nj2 = binsum[j2] - dm + dv
                    if (nj <= target[j]).all() and (nj2 <= target[j2]).all():
                        assign[n], assign[m] = j2, j
                        binsum[j] = nj
                        binsum[j2] = nj2
                        over = binsum[j, b] - target[j, b]
                        moved += 1
                        done = True
                        break
                if not done:
                    break
        if moved == 0:
            break
    return assign, binsum


def prep(x, edge_index, cfg=CFG):
    """Host-side sharding/schedule. Returns (schedule, per-core input maps,
    host metadata for unsharding)."""
    C, NB, WN, NSH, BS, CALL = cfg.C, cfg.NB, cfg.WN, cfg.NSH, cfg.BS, cfg.CALL
    src = np.asarray(edge_index[0]).astype(np.int64)
    dst = np.asarray(edge_index[1]).astype(np.int64)
    E = src.shape[0]

    deg = np.bincount(dst, minlength=cfg.N).astype(np.float64)
    invdeg = (1.0 / np.maximum(deg, 1.0)).astype(np.float32)
    vedge_all = invdeg[dst].astype(bfnp)

    ecore = dst // NSH
    eblock = src // BS

    # --- per-core balance: node-local id -> (window, pos)
    win_of = np.zeros(cfg.N, np.int64)   # window within core
    pos_of = np.zeros(cfg.N, np.int64)   # slot within window
    counts = np.zeros((C, WN, NB), np.int64)
    for c in range(C):
        lo = c * NSH
        dnb = np.zeros((NSH, NB), np.int64)
        emask = ecore == c
        np.add.at(dnb, (dst[emask] - lo, eblock[emask]), 1)
        assign, binsum = _balance_core(dnb, WN)
        # order bins by descending per-block chunk tuple so heavy cells align
        # at the same window index across cores
        kt = np.ceil(binsum / 128).astype(np.int64)
        key = [tuple(-kt[j]) + tuple(-binsum[j]) for j in range(WN)]
        order = sorted(range(WN), key=lambda j: key[j])
        rank = np.empty(WN, np.int64)
        rank[order] = np.arange(WN)
        w = rank[assign]
        win_of[lo:lo + NSH] = w
        # position within window: stable by node id
        order2 = np.lexsort((np.arange(NSH), w))
        pos = np.zeros(NSH, np.int64)
        pcount = np.zeros(WN, np.int64)
        for m in order2:
            pos[m] = pcount[w[m]]
            pcount[w[m]] += 1
        pos_of[lo:lo + NSH] = pos
        cnt = np.zeros((WN, NB), np.int64)
        np.add.at(cnt, (w[dst[emask] - lo], eblock[emask]), 1)
        counts[c] = cnt

    K = np.ceil(counts / 128).astype(np.int64).max(axis=0)  # [WN, NB]
    TCH = int(K.sum())  # chunks per core per layer

    # per-block stream lengths and call counts (uniform across cores)
    Sb = (K.sum(axis=0) * 128).astype(np.int64)             # [NB] idx slots
    ncalls = np.ceil(Sb / CALL).astype(np.int64)
    lastvalid = Sb - (ncalls - 1) * CALL                     # valid idxs in last call

    ewin = win_of[dst]
    epos = pos_of[dst]

    in_maps = []
    for c in range(C):
        lo = c * NSH
        emask = ecore == c
        es, ed = src[emask], dst[emask]
        ew, eb = ewin[emask], eblock[emask]
        ep = epos[emask]
        ev = vedge_all[emask]
        okey = np.lexsort((np.arange(es.shape[0]), eb, ew))
        es, ed, ew, eb, ep, ev = (a[okey] for a in (es, ed, ew, eb, ep, ev))
        cellcnt = counts[c]

        gstreams = [np.zeros(int(ncalls[b]) * CALL, np.int16) for b in range(NB)]
        for b in range(NB):
            if Sb[b] < ncalls[b] * CALL:
                gstreams[b][Sb[b]:] = -1  # tail of last call: skipped by DMA
        sval = np.zeros((TCH, 128, 128), bfnp)  # per-chunk one-hot*invdeg
        drel = np.full((TCH, 128), 255.0, np.float32)
        vval = np.zeros((TCH, 128), np.float32)

        # walk cells in (w, b) order; edges already sorted that way
        eoff = 0
        gcol = 0
        posb = np.zeros(NB, np.int64)
        for w in range(WN):
            for b in range(NB):
                L = int(cellcnt[w, b])
                kwb = int(K[w, b])
                if kwb == 0:
                    assert L == 0
                    continue
                sl = slice(eoff, eoff + L)
                st = int(posb[b])
                gstreams[b][st:st + L] = (es[sl] - b * BS).astype(np.int16)
                # pads within the cell keep idx 0 (gather row 0, killed by
                # one-hot 255)
                ar = np.arange(L)
                sval[gcol + ar // 128, ar % 128, ep[sl]] = ev[sl]
                drel.reshape(-1)[gcol * 128:gcol * 128 + L] = ep[sl]
                vval.reshape(-1)[gcol * 128:gcol * 128 + L] = ev[sl].astype(np.float32)
                posb[b] += kwb * 128
                gcol += kwb
                eoff += L
        assert eoff == es.shape[0]
        assert gcol == TCH

        # wrap gather idx streams into [128, cols] int16 (16-partition wrap,
        # replicated x8)
        gparts = []
        for b in range(NB):
            arr = gstreams[b].reshape(-1, 16).T  # [16, Sb_pad/16]
            gparts.append(arr)
        gidx16 = np.concatenate(gparts, axis=1)          # [16, GCOLS]
        gidx = np.tile(gidx16, (8, 1)).astype(np.int16)  # [128, GCOLS]

        # scatter indices: slot s=(w*128+p) -> node-local row, dummies to trash
        sl_nodes = np.full(cfg.SLOTS, -1, np.int64)
        msk = np.arange(cfg.N)[lo:lo + NSH]
        sl_idx = win_of[msk] * 128 + pos_of[msk]
        sl_nodes[sl_idx] = np.arange(NSH)
        trash = cfg.NSH + (np.arange(cfg.SLOTS) % cfg.TRASH)
        starg = np.where(sl_nodes >= 0, sl_nodes, trash).astype(np.int16)
        sidx16 = starg.reshape(-1, 16).T
        sidx = np.tile(sidx16, (8, 1)).astype(np.int16)

        # xT in slot order
        xT = np.zeros((cfg.D, cfg.SLOTS), bfnp)
        xT[:, sl_idx] = np.asarray(x[lo:lo + NSH]).astype(bfnp).T

        sv = sval
        in_maps.append(dict(
            gidx=gidx, sidx=sidx,
            sval=np.ascontiguousarray(sv.transpose(1, 0, 2).reshape(128, -1)),
            dstrel=np.ascontiguousarray(drel.T), vval=np.ascontiguousarray(vval.T),
            xT=np.ascontiguousarray(xT),
            slot_nodes=sl_nodes,                   # host-only
        ))

    is_dve = np.zeros(TCH, bool)
    sidx_of = np.arange(TCH)
    sched = dict(K=K, TCH=TCH, ncalls=ncalls, lastvalid=lastvalid, Sb=Sb,
                 is_dve=is_dve, sidx_of=sidx_of, NSV=TCH)
    return sched, in_maps


# ---------------------------------------------------------------- program


def build(cfg, sched):
    K, TCH = sched["K"], sched["TCH"]
    ncalls, lastvalid = sched["ncalls"], sched["lastvalid"]
    C, D, NB, WN, NSH, BS = cfg.C, cfg.D, cfg.NB, cfg.WN, cfg.NSH, cfg.BS
    CALL, CALLCH, SLOTS = cfg.CALL, cfg.CALLCH, cfg.SLOTS
    GCOLS_B = [int(ncalls[b]) * (CALL // 16) for b in range(NB)]
    GOFF = np.concatenate([[0], np.cumsum(GCOLS_B)]).astype(int)

    nc = bacc.Bacc(None, num_devices=C, num_swdge_queues=4,
                   dynamic_dma_scratch_size=32768)
    x_d = nc.dram_tensor("xbf", [cfg.N, D], BF, kind="ExternalInput")
    xT_d = nc.dram_tensor("xT", [D, SLOTS], BF, kind="ExternalInput")
    gidx_d = nc.dram_tensor("gidx", [128, int(GOFF[-1])], I16, kind="ExternalInput")
    sidx_d = nc.dram_tensor("sidx", [128, SLOTS // 16], I16, kind="ExternalInput")
    is_dve, sidx_of, NSV = sched["is_dve"], sched["sidx_of"], sched["NSV"]
    sval_d = nc.dram_tensor("sval", [128, NSV * 128], BF, kind="ExternalInput")
    drel_d = nc.dram_tensor("dstrel", [128, TCH], F32, kind="ExternalInput")
    vval_d = nc.dram_tensor("vval", [128, TCH], F32, kind="ExternalInput")
    w_d = {}
    for nm in ("wlt1", "wrt1", "wlt2", "wrt2"):
        w_d[nm] = nc.dram_tensor(nm, [D, D], BF, kind="ExternalInput")
    b1_d = nc.dram_tensor("b1c", [D, 1], F32, kind="ExternalInput")
    b2_d = nc.dram_tensor("b2r", [1, D], F32, kind="ExternalInput")
    out_d = nc.dram_tensor("out", [SLOTS, D], F32, kind="ExternalOutput")

    ident_d = nc.inline_tensor(np.eye(128, dtype=bfnp), "identc")
    iota_d = nc.inline_tensor(
        np.broadcast_to(np.arange(128, dtype=bfnp), (128, 128)).copy(), "iotac")
    ones_d = nc.inline_tensor(np.ones((1, 128), np.float32), "onesc")

    hsh_d = nc.dram_tensor("hshard", [NSH + cfg.TRASH, D], BF)  # Internal
    hfull_d = nc.dram_tensor("hfull", [cfg.N, D], BF, addr_space="Shared")

    with tile.TileContext(nc) as tc, ExitStack() as ctx:
        const = ctx.enter_context(tc.tile_pool(name="const", bufs=1))
        meta = ctx.enter_context(tc.tile_pool(name="meta", bufs=1))
        gpool = ctx.enter_context(tc.tile_pool(name="gather", bufs=6))
        spool = ctx.enter_context(tc.tile_pool(name="sv", bufs=6))
        ohp = ctx.enter_context(tc.tile_pool(name="oh", bufs=8))
        mwp = ctx.enter_context(tc.tile_pool(name="mw", bufs=4))
        htp = ctx.enter_context(tc.tile_pool(name="ht", bufs=1))
        xtp = ctx.enter_context(tc.tile_pool(name="xt", bufs=1))
        stgp = ctx.enter_context(tc.tile_pool(name="stg", bufs=2))
        psA = ctx.enter_context(tc.tile_pool(name="psA", bufs=3, space="PSUM"))
        psB = ctx.enter_context(tc.tile_pool(name="psB", bufs=2, space="PSUM"))
        psT = ctx.enter_context(tc.tile_pool(name="psT", bufs=2, space="PSUM"))

        def load(pool, dram, shape, dtype):
            t = pool.tile(shape, dtype, tag=dram.name)
            nc.sync.dma_start(t[:], dram[:])
            return t

        ident_s = load(const, ident_d, [128, 128], BF)
        iota_s = load(const, iota_d, [128, 128], BF)
        drel_s = load(meta, drel_d, [128, TCH], F32)
        vval_s = load(meta, vval_d, [128, TCH], F32)
        ones_s = load(const, ones_d, [1, 128], F32)
        w_s = {nm: load(const, w_d[nm], [D, D], BF) for nm in w_d}
        b1_s = load(const, b1_d, [D, 1], F32)
        b2_s = load(const, b2_d, [1, D], F32)

        gidx_s = load(meta, gidx_d, [128, int(GOFF[-1])], I16)
        sidx_s = load(meta, sidx_d, [128, SLOTS // 16], I16)
        xT_s = load(xtp, xT_d, [D, SLOTS], BF)
        hT_s = htp.tile([D, SLOTS], BF, tag="hT")

        # zero h_shard
        zt = const.tile([128, 1024], BF, tag="zeros")
        nc.vector.memset(zt[:], 0.0)
        nrows_total = NSH + cfg.TRASH
        r0 = 0
        while r0 < nrows_total:
            nr = min(1024, nrows_total - r0)
            nc.sync.dma_start(hsh_d[r0:r0 + nr, :], zt[:, :nr])
            r0 += nr

        def run_layer(L):
            src_d = x_d if L == 1 else hfull_d
            posb = [0] * NB
            gt = [None] * NB
            stg_tile = None
            ostg_tile = None
            SCW, OCW = cfg.SCW, cfg.OCW
            for w in range(WN):
                nchunks_w = int(K[w].sum())
                psum_a = psA.tile([128, 128], F32, tag="agg")
                ci = 0
                for b in range(NB):
                    for k in range(int(K[w, b])):
                        pos = posb[b]
                        call_i, col = divmod(pos, CALLCH)
                        if col == 0:
                            gt[b] = gpool.tile([128, CALLCH, 128], BF, tag=f"g{b}",
                                               name=f"g{b}_{L}_{call_i}")
                            nvalid = CALL if call_i < int(ncalls[b]) - 1 else int(lastvalid[b])
                            ioff = GOFF[b] + call_i * (CALL // 16)
                            nc.gpsimd.dma_gather(
                                out_ap=gt[b][:],
                                in_ap=src_d[b * BS:(b + 1) * BS, :],
                                idxs_ap=gidx_s[:, ioff:ioff + CALL // 16],
                                num_idxs=CALL,
                                num_idxs_reg=nvalid,
                                elem_size=D,
                            )
                        gcol = run_layer.gcol
                        if is_dve[gcol]:
                            S = ohp.tile([128, 128], BF, tag="oh",
                                         name=f"oh{L}_{gcol}")
                            nc.vector.tensor_scalar(
                                out=S[:], in0=iota_s[:],
                                scalar1=drel_s[:, gcol:gcol + 1],
                                scalar2=vval_s[:, gcol:gcol + 1],
                                op0=mybir.AluOpType.is_equal,
                                op1=mybir.AluOpType.mult,
                            )
                            rhs_ap = S[:]
                        else:
                            si, sc = divmod(int(sidx_of[gcol]), 8)
                            if sc == 0:
                                nch = min(8, NSV - si * 8)
                                st = spool.tile([128, 8, 128], BF, tag="sv",
                                                name=f"sv{L}_{si}")
                                nc.sync.dma_start(
                                    st[:, :nch, :],
                                    sval_d[:, si * 1024:si * 1024 + nch * 128])
                                run_layer.stile = st
                            rhs_ap = run_layer.stile[:, sc, :]
                        nc.tensor.matmul(
                            out=psum_a[:], lhsT=gt[b][:, col, :], rhs=rhs_ap,
                            start=(ci == 0), stop=(ci == nchunks_w - 1),
                        )
                        run_layer.gcol += 1
                        posb[b] += 1
                        ci += 1
                m_s = mwp.tile([128, 128], BF, tag="mw")
                if nchunks_w:
                    nc.vector.tensor_copy(m_s[:], psum_a[:])
                else:
                    nc.vector.memset(m_s[:], 0.0)
                wsl = slice(w * 128, (w + 1) * 128)
                if L == 1:
                    psum_h = psB.tile([128, 128], F32, tag="h")
                    nc.tensor.matmul(out=psum_h[:], lhsT=w_s["wlt1"][:], rhs=m_s[:],
                                     start=True, stop=False)
                    nc.tensor.matmul(out=psum_h[:], lhsT=w_s["wrt1"][:],
                                     rhs=xT_s[:, wsl], start=False, stop=True)
                    nc.scalar.activation(hT_s[:, wsl], psum_h[:],
                                         mybir.ActivationFunctionType.Identity,
                                         bias=b1_s[:, 0:1], scale=1.0)
                    psum_t = psT.tile([128, 128], BF, tag="tr")
                    nc.tensor.transpose(psum_t[:], hT_s[:, wsl], ident_s[:])
                    wi = w % SCW
                    if wi == 0:
                        stg_tile = stgp.tile([128, SCW, 128], BF, tag="stg")
                    nc.vector.tensor_copy(stg_tile[:, wi, :], psum_t[:])
                    if wi == SCW - 1 or w == WN - 1:
                        used = wi + 1
                        w0 = w - wi
                        nc.gpsimd.dma_scatter_add(
                            out_ap=hsh_d[:, :],
                            in_ap=stg_tile[:, :used, :],
                            idxs_ap=sidx_s[:, w0 * 8:(w0 + used) * 8],
                            num_idxs=used * 128,
                            num_idxs_reg=used * 128,
                            elem_size=D,
                        )
                else:
                    psum_h = psB.tile([128, 128], F32, tag="h")
                    nc.tensor.matmul(out=psum_h[:], lhsT=m_s[:], rhs=w_s["wlt2"][:],
                                     start=True, stop=False)
                    nc.tensor.matmul(out=psum_h[:], lhsT=hT_s[:, wsl],
                                     rhs=w_s["wrt2"][:], start=False, stop=False)
                    nc.tensor.matmul(out=psum_h[:], lhsT=ones_s[0:1, :],
                                     rhs=b2_s[0:1, :], start=False, stop=True)
                    wi = w % OCW
                    if wi == 0:
                        ostg_tile = stgp.tile([128, OCW, 128], F32, tag="ostg")
                    nc.vector.tensor_copy(ostg_tile[:, wi, :], psum_h[:])
                    if wi == OCW - 1 or w == WN - 1:
                        used = wi + 1
                        w0 = w - wi
                        oap = out_d[:].rearrange("(w p) f -> p w f", p=128)
                        nc.sync.dma_start(oap[:, w0:w0 + used, :],
                                          ostg_tile[:, :used, :])

        run_layer.gcol = 0
        run_layer.stile = None
        run_layer(1)
        nc.gpsimd.collective_compute(
            "AllGather", mybir.AluOpType.bypass,
            replica_groups=[list(range(C))],
            ins=[hsh_d[0:NSH, :]],
            outs=[hfull_d[:]],
        )
        run_layer.gcol = 0
        run_layer.stile = None
        run_layer(2)

    # spread SWDGE gather/scatter descriptor generation across the 4 SWDGE
    # queues (parallel Q7 pairs). Tile assigned DMASW lanes round-robin in
    # scheduled order; keep sem-lane <-> queue binding consistent by deriving
    # the queue from the lane (lane % 4).
    from concourse.tile_sem_assignment import PROC_NAME_TO_IDX
    dmasw0 = PROC_NAME_TO_IDX["DMASW0"]
    for inst in nc.inst_map.values():
        if isinstance(inst, (mybir.InstDMAGatherAnt, mybir.InstDMAScatterAddAnt)):
            proc = getattr(inst, "bass_scheduled_proc", None)
            if proc is not None and dmasw0 <= proc < dmasw0 + 8:
                inst.queue_num = (proc - dmasw0) % 4

    nc.compile()
    return nc


# ---------------------------------------------------------------- kernel


def kernel(**inputs):
    cfg = CFG
    x = np.asarray(inputs["x"], np.float32)
    ei = np.asarray(inputs["edge_index"])
    sched, in_maps = prep(x, ei, cfg)
    nc = build(cfg, sched)

    x_bf = x.astype(bfnp)
    shared = dict(
        xbf=x_bf,
        wlt1=np.ascontiguousarray(np.asarray(inputs["Wl1"], np.float32).T.astype(bfnp)),
        wrt1=np.ascontiguousarray(np.asarray(inputs["Wr1"], np.float32).T.astype(bfnp)),
        wlt2=np.ascontiguousarray(np.asarray(inputs["Wl2"], np.float32).T.astype(bfnp)),
        wrt2=np.ascontiguousarray(np.asarray(inputs["Wr2"], np.float32).T.astype(bfnp)),
        b1c=np.asarray(inputs["b1"], np.float32).reshape(cfg.D, 1).copy(),
        b2r=np.asarray(inputs["b2"], np.float32).reshape(1, cfg.D).copy(),
    )
    slot_nodes = [m.pop("slot_nodes") for m in in_maps]
    run_maps = [dict(shared, **{k: v for k, v in m.items()}) for m in in_maps]

    res = run_bass_kernel_spmd(nc, run_maps, core_ids=list(range(cfg.C)))
    out = np.empty((cfg.N, cfg.D), np.float32)
    for c in range(cfg.C):
        oc = res.results[c]["out"]
        sn = slot_nodes[c]
        real = sn >= 0
        out[c * cfg.NSH + sn[real]] = oc[real]
    return out


if __name__ == "__main__":
    d = np.load("/tmp/inputs.npz")
    ins = {k: d[k] for k in ("x", "edge_index", "Wl1", "Wr1", "b1", "Wl2", "Wr2", "b2")}
    got = kernel(**ins)
    exp = d["expected"]
    err = np.abs(got - exp).max() / np.abs(exp).max()
    print("Relative error:", err)
